# revision 6
# baseline (speedup 1.0000x reference)
"""Local-window GQA attention on 8 trn2 NeuronCores.

Sharding: sequence-parallel. Core c owns queries [c*512, (c+1)*512) and
redundantly computes K/V for its 1024-position key buffer
[c*512-512, c*512+512) from x (halo recompute instead of any collective).
All projections, RoPE, banded-causal softmax and o_proj run on-device in
fp32/fp32r; host only shards/transposes inputs and concatenates outputs.

Device-side layouts are transposed ([feature, time]) so every matmul
contraction lands on the partition axis without on-chip transposes of
activations; only the post-softmax P tiles are transposed (PE transpose).
RoPE rotate-half is a PERM-matrix matmul (DVE cannot cross partitions);
softmax runs unnormalized (scaled scores stay within +-~8 so exp cannot
overflow) with the normalization folded into P, and core 0's halo keys are
handled by zeroed x + a per-core additive Z correction (halo keys give
exp(0)=1, subtracted from Z; their v contribution is 0).
"""

import os

import numpy as np

Q_HEADS, KV_HEADS, HEAD_DIM, WINDOW = 16, 4, 128, 512
HIDDEN = 2048
T_FULL = 4096
NCORES = 8
CHUNK = T_FULL // NCORES  # 512
KBUF = CHUNK + WINDOW  # 1024
REPS = Q_HEADS // KV_HEADS  # 4
HALF = HEAD_DIM // 2  # 64
SCALE = 1.0 / np.sqrt(HEAD_DIM)
NEG = -30000.0
ROPE_BASE = 10000.0

_CACHE = {}
LAST_RESULTS = None


def _install_profhook():
    """Make run_bass_kernel_spmd(trace=True) work under axon (the agent
    image's antenv lacks axon_hooks; register the ctypes NTFF hook)."""
    import sys
    import types

    if "antenv.axon_hooks" in sys.modules:
        return
    m = types.ModuleType("antenv.axon_hooks")
    hook = [None]
    m.set_axon_ntff_profile_hook = lambda h: hook.__setitem__(0, h)
    m.get_axon_ntff_profile_hook = lambda: hook[0]
    sys.modules["antenv.axon_hooks"] = m
    try:
        import antenv

        antenv.axon_hooks = m
        from trn_agent_boot.trn_boot import _ntff_profile_via_ctypes

        m.set_axon_ntff_profile_hook(
            _ntff_profile_via_ctypes("/opt/axon/libaxon_pjrt.so")
        )
    except Exception:
        pass


def _build():
    import concourse.bacc as bacc
    import concourse.mybir as mybir
    from concourse import tile

    f32 = mybir.dt.float32
    f32r = mybir.dt.float32r
    Exp = mybir.ActivationFunctionType.Exp
    mult = mybir.AluOpType.mult
    add = mybir.AluOpType.add
    subtract = mybir.AluOpType.subtract

    nc = bacc.Bacc(None, target_bir_lowering=False)

    # ---- DRAM I/O (per core) ----
    xkvT = nc.dram_tensor("xkvT", [HIDDEN, KBUF], f32r, kind="ExternalInput")
    wqT = nc.dram_tensor("wqT", [HIDDEN, 2048], f32r, kind="ExternalInput")
    wkT = nc.dram_tensor("wkT", [HIDDEN, 512], f32r, kind="ExternalInput")
    wvT = nc.dram_tensor("wvT", [HIDDEN, 512], f32r, kind="ExternalInput")
    woT = nc.dram_tensor("woT", [2048, 2048], f32r, kind="ExternalInput")
    cosfk = nc.dram_tensor("cosfk", [128, KBUF], f32, kind="ExternalInput")
    sinfk = nc.dram_tensor("sinfk", [128, KBUF], f32, kind="ExternalInput")
    cosfq = nc.dram_tensor("cosfq", [128, 2 * CHUNK], f32, kind="ExternalInput")
    sinfq = nc.dram_tensor("sinfq", [128, 2 * CHUNK], f32, kind="ExternalInput")
    perm_d = nc.dram_tensor("perm", [128, 128], f32r, kind="ExternalInput")
    ident_d = nc.dram_tensor("ident", [128, 128], f32r, kind="ExternalInput")
    band_d = nc.dram_tensor("band", [128, 2, 128], f32, kind="ExternalInput")
    zcor_d = nc.dram_tensor("zcor", [128, 4], f32, kind="ExternalInput")
    y = nc.dram_tensor("y", [CHUNK, HIDDEN], f32, kind="ExternalOutput")

    NT = HIDDEN // 128  # 16 h-tiles

    with tile.TileContext(nc) as tc:
        with (
            tc.tile_pool(name="persist", bufs=1) as pp,
            tc.tile_pool(name="consts", bufs=1) as cp,
        ):
            # persistent activations
            v_all = pp.tile([128, 8, 512], f32r)  # [s, s-tile, d(4 heads)]
            oT_all = pp.tile([128, 16, 512], f32r)  # [d, head, t]
            # constants
            cosk_t = cp.tile([128, KBUF], f32)
            sink_t = cp.tile([128, KBUF], f32)
            cosq_t = cp.tile([128, 2 * CHUNK], f32)
            sinq_t = cp.tile([128, 2 * CHUNK], f32)
            perm_t = cp.tile([128, 128], f32r)
            ident_t = cp.tile([128, 128], f32r)
            band_t = cp.tile([128, 2, 128], f32)
            zcor_t = cp.tile([128, 4], f32)
            nc.sync.dma_start(out=cosk_t[:], in_=cosfk[:])
            nc.sync.dma_start(out=sink_t[:], in_=sinfk[:])
            nc.sync.dma_start(out=cosq_t[:], in_=cosfq[:])
            nc.sync.dma_start(out=sinq_t[:], in_=sinfq[:])
            nc.sync.dma_start(out=perm_t[:], in_=perm_d[:])
            nc.sync.dma_start(out=ident_t[:], in_=ident_d[:])
            nc.sync.dma_start(out=band_t[:], in_=band_d[:])
            nc.sync.dma_start(out=zcor_t[:], in_=zcor_d[:])

            with tc.tile_pool(name="xkvp", bufs=1) as xp:
                xkv_t = xp.tile([128, NT, KBUF], f32r)
                for hi in range(NT):
                    nc.sync.dma_start(
                        out=xkv_t[:, hi, :],
                        in_=xkvT[hi * 128 : (hi + 1) * 128, :],
                    )

                # ---- v projection (all 4 kv heads at once, t-tile major) ----
                with (
                    tc.tile_pool(name="wvp", bufs=1) as wvp,
                    tc.tile_pool(name="vps", bufs=1, space="PSUM") as vps,
                ):
                    wv_t = wvp.tile([128, NT, 512], f32r)
                    for hi in range(NT):
                        nc.sync.dma_start(
                            out=wv_t[:, hi, :],
                            in_=wvT[hi * 128 : (hi + 1) * 128, :],
                        )
                    for ts in range(8):
                        pv = vps.tile([128, 512], f32, tag="pv", bufs=4)
                        for hi in range(NT):
                            nc.tensor.matmul(
                                pv[:],
                                xkv_t[:, hi, ts * 128 : (ts + 1) * 128],
                                wv_t[:, hi, :],
                                start=(hi == 0),
                                stop=(hi == NT - 1),
                            )
                        nc.scalar.copy(out=v_all[:, ts, :], in_=pv[:])

                # ---- per kv-group: k proj+rope, q proj+rope, attention ----
                with (
                    tc.tile_pool(name="gw", bufs=1) as gw,
                    tc.tile_pool(name="gact", bufs=1) as ga,
                    tc.tile_pool(name="gps", bufs=1, space="PSUM") as gp,
                ):
                    for g in range(KV_HEADS):
                        # -- k projection: kT [d=128, t=KBUF] --
                        wk_g = gw.tile([128, NT, 128], f32r, tag="wk", bufs=2)
                        nc.sync.dma_start(
                            out=wk_g[:],
                            in_=wkT.rearrange("(a p) o -> p a o", p=128)[
                                :, :, g * 128 : (g + 1) * 128
                            ],
                        )
                        pk = gp.tile([128, 1024], f32, tag="big", bufs=2)
                        for th in range(2):
                            sl = slice(th * 512, (th + 1) * 512)
                            for hi in range(NT):
                                nc.tensor.matmul(
                                    pk[:, sl],
                                    wk_g[:, hi, :],
                                    xkv_t[:, hi, sl],
                                    start=(hi == 0),
                                    stop=(hi == NT - 1),
                                )
                        kraw = ga.tile([128, 1024], f32r, tag="raw", bufs=2)
                        nc.scalar.copy(out=kraw[:], in_=pk[:])
                        khat = gp.tile([128, 1024], f32, tag="big", bufs=2)
                        for th in range(2):
                            sl = slice(th * 512, (th + 1) * 512)
                            nc.tensor.matmul(
                                khat[:, sl], perm_t[:], kraw[:, sl],
                                start=True, stop=True,
                            )
                        kT_g = ga.tile([128, KBUF], f32r, tag="kT", bufs=1)
                        atmp = ga.tile([128, 1024], f32, tag="tmp", bufs=2)
                        nc.vector.tensor_tensor(
                            out=atmp[:], in0=kraw[:], in1=cosk_t[:], op=mult
                        )
                        nc.vector.tensor_tensor(
                            out=kT_g[:], in0=khat[:], in1=sink_t[:], op=mult
                        )
                        nc.vector.tensor_tensor(
                            out=kT_g[:], in0=kT_g[:], in1=atmp[:], op=add
                        )

                        # -- q projection for heads 4g..4g+3, 2 pairs, hi-outer --
                        pqs = [
                            gp.tile([128, 1024], f32, tag="big", bufs=2,
                                    name=f"pq_{g}_{p}")
                            for p in range(2)
                        ]
                        for hi in range(NT):
                            wq_t = gw.tile(
                                [128, 512], f32r, tag="wq", bufs=4,
                                name=f"wq_{g}_{hi}",
                            )
                            nc.sync.dma_start(
                                out=wq_t[:],
                                in_=wqT[
                                    hi * 128 : (hi + 1) * 128,
                                    g * 512 : (g + 1) * 512,
                                ],
                            )
                            for pair in range(2):
                                for j in range(2):
                                    sl = slice(j * 512, (j + 1) * 512)
                                    nc.tensor.matmul(
                                        pqs[pair][:, sl],
                                        wq_t[:, (2 * pair + j) * 128
                                             : (2 * pair + j + 1) * 128],
                                        xkv_t[:, hi, 512:1024],
                                        start=(hi == 0),
                                        stop=(hi == NT - 1),
                                    )
                        qT_g = ga.tile([128, 2, 1024], f32r, tag="qT", bufs=1)
                        for pair in range(2):
                            qraw = ga.tile([128, 1024], f32r, tag="raw", bufs=2,
                                           name=f"qraw_{g}_{pair}")
                            nc.scalar.copy(out=qraw[:], in_=pqs[pair][:])
                            qhat = gp.tile([128, 1024], f32, tag="big", bufs=2,
                                           name=f"qhat_{g}_{pair}")
                            for j in range(2):
                                sl = slice(j * 512, (j + 1) * 512)
                                nc.tensor.matmul(
                                    qhat[:, sl], perm_t[:], qraw[:, sl],
                                    start=True, stop=True,
                                )
                            btmp = ga.tile([128, 1024], f32, tag="tmp", bufs=2,
                                           name=f"btmp_{g}_{pair}")
                            nc.vector.tensor_tensor(
                                out=btmp[:], in0=qraw[:], in1=cosq_t[:], op=mult
                            )
                            nc.vector.tensor_tensor(
                                out=qT_g[:, pair, :], in0=qhat[:], in1=sinq_t[:],
                                op=mult,
                            )
                            nc.vector.tensor_tensor(
                                out=qT_g[:, pair, :], in0=qT_g[:, pair, :],
                                in1=btmp[:], op=add,
                            )

                        # -- attention for the 4 heads of this group --
                        for hl in range(REPS):
                            qh = 4 * g + hl
                            pair, j = hl // 2, hl % 2
                            for qt in range(4):
                                s_ps = gp.tile(
                                    [128, 640], f32, tag="S", bufs=2,
                                    name=f"s_ps_{qh}_{qt}",
                                )
                                q_sl = qT_g[
                                    :, pair,
                                    j * 512 + qt * 128 : j * 512 + (qt + 1) * 128,
                                ]
                                nc.tensor.matmul(
                                    s_ps[:, 0:512],
                                    q_sl,
                                    kT_g[:, qt * 128 : qt * 128 + 512],
                                    start=True,
                                    stop=True,
                                )
                                nc.tensor.matmul(
                                    s_ps[:, 512:640],
                                    q_sl,
                                    kT_g[:, qt * 128 + 512 : qt * 128 + 640],
                                    start=True,
                                    stop=True,
                                )
                                # band mask on first/last 128-col blocks
                                s_v = s_ps[:].rearrange("p (a b) -> p a b", b=128)
                                nc.vector.tensor_tensor(
                                    out=s_v[:, 0::4, :],
                                    in0=s_v[:, 0::4, :],
                                    in1=band_t[:],
                                    op=add,
                                )
                                p_sb = ga.tile(
                                    [128, 640], f32r, tag="P", bufs=3,
                                    name=f"p_sb_{qh}_{qt}",
                                )
                                z_h = ga.tile([128, 1], f32, tag="z", bufs=3,
                                              name=f"z_{qh}_{qt}")
                                nc.scalar.activation(
                                    out=p_sb[:], in_=s_ps[:], func=Exp,
                                    accum_out=z_h[:],
                                )
                                r_h = ga.tile([128, 1], f32, tag="r", bufs=3,
                                              name=f"r_{qh}_{qt}")
                                nc.vector.tensor_tensor(
                                    out=r_h[:], in0=z_h[:],
                                    in1=zcor_t[:, qt : qt + 1], op=subtract
                                )
                                nc.vector.reciprocal(out=r_h[:], in_=r_h[:])
                                nc.gpsimd.tensor_scalar(
                                    out=p_sb[:], in0=p_sb[:],
                                    scalar1=r_h[:], scalar2=None,
                                    op0=mult,
                                )
                                pt_ps = gp.tile(
                                    [128, 5, 128], f32r, tag="S", bufs=2,
                                    name=f"pt_ps_{qh}_{qt}",
                                )
                                for i in range(5):
                                    nc.tensor.transpose(
                                        pt_ps[:, i, :],
                                        p_sb[:, i * 128 : (i + 1) * 128],
                                        ident_t[:],
                                    )
                                pt_sb = ga.tile(
                                    [128, 5, 128], f32r, tag="PT", bufs=2,
                                    name=f"pt_sb_{qh}_{qt}",
                                )
                                nc.scalar.copy(out=pt_sb[:], in_=pt_ps[:])
                                o_ps = gp.tile(
                                    [128, 128], f32, tag="S", bufs=2,
                                    name=f"o_ps_{qh}_{qt}",
                                )
                                for i in range(5):
                                    nc.tensor.matmul(
                                        o_ps[:],
                                        v_all[:, qt + i, g * 128 : (g + 1) * 128],
                                        pt_sb[:, i, :],
                                        start=(i == 0),
                                        stop=(i == 4),
                                    )
                                nc.scalar.copy(
                                    out=oT_all[:, qh, qt * 128 : (qt + 1) * 128],
                                    in_=o_ps[:],
                                )

            # ---- o_proj: y[t, :] = sum_o oT[o, t] * woT[o, :] ----
            with (
                tc.tile_pool(name="wop", bufs=1) as wop,
                tc.tile_pool(name="yp", bufs=1) as yp,
                tc.tile_pool(name="ops", bufs=1, space="PSUM") as ops,
            ):
                for half in range(2):
                    pys = [
                        ops.tile([128, 1024], f32, tag="py", bufs=4,
                                 name=f"py_{half}_{qt}")
                        for qt in range(4)
                    ]
                    for oh in range(NT):
                        wo_t = wop.tile([128, 1024], f32r, tag="wo", bufs=5,
                                        name=f"wo_{half}_{oh}")
                        nc.sync.dma_start(
                            out=wo_t[:],
                            in_=woT[
                                oh * 128 : (oh + 1) * 128,
                                half * 1024 : (half + 1) * 1024,
                            ],
                        )
                        for qt in range(4):
                            for c in range(2):
                                sl = slice(c * 512, (c + 1) * 512)
                                nc.tensor.matmul(
                                    pys[qt][:, sl],
                                    oT_all[:, oh, qt * 128 : (qt + 1) * 128],
                                    wo_t[:, sl],
                                    start=(oh == 0),
                                    stop=(oh == NT - 1),
                                )
                    for qt in range(4):
                        ysb = yp.tile([128, 1024], f32, tag="y", bufs=4,
                                      name=f"y_{half}_{qt}")
                        nc.scalar.copy(out=ysb[:], in_=pys[qt][:])
                        nc.sync.dma_start(
                            out=y[
                                qt * 128 : (qt + 1) * 128,
                                half * 1024 : (half + 1) * 1024,
                            ],
                            in_=ysb[:],
                        )

    nc.compile()
    return nc


def _host_prep(x, wq, wk, wv, wo):
    x = np.asarray(x, dtype=np.float32)
    B, T, H = x.shape
    assert (B, T, H) == (1, T_FULL, HIDDEN)
    xf = x[0]

    inv_freq = (
        1.0 / (ROPE_BASE ** (np.arange(HALF, dtype=np.float32) / HALF))
    ).astype(np.float32)
    t_all = np.arange(T_FULL, dtype=np.float32)
    freqs = np.outer(t_all, inv_freq).astype(np.float32)  # [T, 64]
    cos_all = np.cos(freqs).astype(np.float32)
    sin_all = np.sin(freqs).astype(np.float32)

    wqT = np.ascontiguousarray(np.asarray(wq, np.float32).T)
    wkT = np.ascontiguousarray(np.asarray(wk, np.float32).T)
    wvT = np.ascontiguousarray(np.asarray(wv, np.float32).T)
    woT = np.ascontiguousarray(np.asarray(wo, np.float32).T)

    perm = np.zeros((128, 128), np.float32)
    for d in range(HALF):
        perm[HALF + d, d] = -1.0  # khat[d] = -k[d+64]
        perm[d, HALF + d] = 1.0  # khat[d+64] = k[d]
    ident = np.eye(128, dtype=np.float32)

    band = np.zeros((128, 2, 128), np.float32)
    r = np.arange(128)[:, None]
    jj = np.arange(128)[None, :]
    band[:, 0, :] = np.where(jj >= r, 0.0, NEG)
    band[:, 1, :] = np.where(jj <= r, 0.0, NEG)

    in_maps = []
    for c in range(NCORES):
        S = c * CHUNK
        lo = S - WINDOW
        xkv = np.zeros((KBUF, HIDDEN), np.float32)
        ck = np.ones((KBUF, HALF), np.float32)  # cos=1 where pos undefined
        sk = np.zeros((KBUF, HALF), np.float32)
        src_lo = max(0, lo)
        off = src_lo - lo
        xkv[off:] = xf[src_lo : S + CHUNK]
        ck[off:] = cos_all[src_lo : S + CHUNK]
        sk[off:] = sin_all[src_lo : S + CHUNK]

        cosfk = np.concatenate([ck.T, ck.T], axis=0)  # [128, KBUF]
        sinfk = np.concatenate([sk.T, sk.T], axis=0)
        cq = (cos_all[S : S + CHUNK] * SCALE).astype(np.float32)
        sq = (sin_all[S : S + CHUNK] * SCALE).astype(np.float32)
        cosfq1 = np.concatenate([cq.T, cq.T], axis=0)  # [128, CHUNK]
        sinfq1 = np.concatenate([sq.T, sq.T], axis=0)
        cosfq = np.concatenate([cosfq1, cosfq1], axis=1)  # [128, 2*CHUNK]
        sinfq = np.concatenate([sinfq1, sinfq1], axis=1)

        zcor = np.zeros((128, 4), np.float32)
        if c == 0:
            i = (np.arange(4) * 128)[None, :] + np.arange(128)[:, None]
            zcor[:] = np.maximum(0, WINDOW - i)

        in_maps.append(
            {
                "xkvT": np.ascontiguousarray(xkv.T),
                "wqT": wqT,
                "wkT": wkT,
                "wvT": wvT,
                "woT": woT,
                "cosfk": np.ascontiguousarray(cosfk),
                "sinfk": np.ascontiguousarray(sinfk),
                "cosfq": np.ascontiguousarray(cosfq),
                "sinfq": np.ascontiguousarray(sinfq),
                "perm": perm,
                "ident": ident,
                "band": band,
                "zcor": zcor,
            }
        )
    return in_maps


def kernel(x, wq, wk, wv, wo):
    global LAST_RESULTS
    from concourse.bass_utils import run_bass_kernel_spmd

    if os.environ.get("BASS_TRACE"):
        _install_profhook()
    if "nc" not in _CACHE:
        _CACHE["nc"] = _build()
    nc = _CACHE["nc"]
    in_maps = _host_prep(x, wq, wk, wv, wo)
    res = run_bass_kernel_spmd(nc, in_maps, core_ids=list(range(NCORES)))
    LAST_RESULTS = res
    out = np.concatenate([res.results[c]["y"] for c in range(NCORES)], axis=0)
    return out[None].astype(np.float32)


# revision 7
# speedup vs baseline: 1.8530x; 1.8530x over previous
"""Local-window GQA attention on 8 trn2 NeuronCores.

Sharding: sequence-parallel. Core c owns queries [c*512, (c+1)*512) and
redundantly computes K/V for its 1024-position key buffer
[c*512-512, c*512+512) from x (halo recompute instead of any collective).
All projections, RoPE, banded-causal softmax and o_proj run on-device in
fp32/fp32r; host only shards/transposes inputs and concatenates outputs.

Device-side layouts are transposed ([feature, time]) so every matmul
contraction lands on the partition axis without on-chip transposes of
activations; only the post-softmax P tiles are transposed (PE transpose).
RoPE rotate-half is a PERM-matrix matmul (DVE cannot cross partitions);
softmax runs unnormalized (scaled scores stay within +-~8 so exp cannot
overflow) with the normalization folded into P, and core 0's halo keys are
handled by zeroed x + a per-core additive Z correction (halo keys give
exp(0)=1, subtracted from Z; their v contribution is 0).
"""

import os

import numpy as np

Q_HEADS, KV_HEADS, HEAD_DIM, WINDOW = 16, 4, 128, 512
HIDDEN = 2048
T_FULL = 4096
NCORES = 8
CHUNK = T_FULL // NCORES  # 512
KBUF = CHUNK + WINDOW  # 1024
REPS = Q_HEADS // KV_HEADS  # 4
HALF = HEAD_DIM // 2  # 64
SCALE = 1.0 / np.sqrt(HEAD_DIM)
NEG = -30000.0
ROPE_BASE = 10000.0

_CACHE = {}
LAST_RESULTS = None


def _install_profhook():
    """Make run_bass_kernel_spmd(trace=True) work under axon (the agent
    image's antenv lacks axon_hooks; register the ctypes NTFF hook)."""
    import sys
    import types

    if "antenv.axon_hooks" in sys.modules:
        return
    m = types.ModuleType("antenv.axon_hooks")
    hook = [None]
    m.set_axon_ntff_profile_hook = lambda h: hook.__setitem__(0, h)
    m.get_axon_ntff_profile_hook = lambda: hook[0]
    sys.modules["antenv.axon_hooks"] = m
    try:
        import antenv

        antenv.axon_hooks = m
        from trn_agent_boot.trn_boot import _ntff_profile_via_ctypes

        m.set_axon_ntff_profile_hook(
            _ntff_profile_via_ctypes("/opt/axon/libaxon_pjrt.so")
        )
    except Exception:
        pass


def _build():
    import concourse.bacc as bacc
    import concourse.mybir as mybir
    from concourse import tile

    f32 = mybir.dt.float32
    f32r = mybir.dt.float32r
    Exp = mybir.ActivationFunctionType.Exp
    mult = mybir.AluOpType.mult
    add = mybir.AluOpType.add
    subtract = mybir.AluOpType.subtract

    nc = bacc.Bacc(None, target_bir_lowering=False)

    # ---- DRAM I/O (per core) ----
    xkvT = nc.dram_tensor("xkvT", [HIDDEN, KBUF], f32r, kind="ExternalInput")
    wqT = nc.dram_tensor("wqT", [HIDDEN, 2048], f32r, kind="ExternalInput")
    wkT = nc.dram_tensor("wkT", [HIDDEN, 512], f32r, kind="ExternalInput")
    wvT = nc.dram_tensor("wvT", [HIDDEN, 512], f32r, kind="ExternalInput")
    woT = nc.dram_tensor("woT", [2048, 2048], f32r, kind="ExternalInput")
    cosfk = nc.dram_tensor("cosfk", [128, KBUF], f32, kind="ExternalInput")
    sinfk = nc.dram_tensor("sinfk", [128, KBUF], f32, kind="ExternalInput")
    cosfq = nc.dram_tensor("cosfq", [128, 2 * CHUNK], f32, kind="ExternalInput")
    sinfq = nc.dram_tensor("sinfq", [128, 2 * CHUNK], f32, kind="ExternalInput")
    perm_d = nc.dram_tensor("perm", [128, 128], f32r, kind="ExternalInput")
    ident_d = nc.dram_tensor("ident", [128, 128], f32r, kind="ExternalInput")
    band_d = nc.dram_tensor("band", [128, 2, 128], f32, kind="ExternalInput")
    zcor_d = nc.dram_tensor("zcor", [128, 4], f32, kind="ExternalInput")
    y = nc.dram_tensor("y", [CHUNK, HIDDEN], f32, kind="ExternalOutput")

    NT = HIDDEN // 128  # 16 h-tiles

    with tile.TileContext(nc) as tc:
        with (
            tc.tile_pool(name="persist", bufs=1) as pp,
            tc.tile_pool(name="consts", bufs=1) as cp,
        ):
            # persistent activations
            v_all = pp.tile([128, 8, 512], f32r)  # [s, s-tile, d(4 heads)]
            oT_all = pp.tile([128, 16, 512], f32r)  # [d, head, t]
            # constants
            cosk_t = cp.tile([128, KBUF], f32)
            sink_t = cp.tile([128, KBUF], f32)
            cosq_t = cp.tile([128, 2 * CHUNK], f32)
            sinq_t = cp.tile([128, 2 * CHUNK], f32)
            perm_t = cp.tile([128, 128], f32r)
            ident_t = cp.tile([128, 128], f32r)
            band_t = cp.tile([128, 2, 128], f32)
            zcor_t = cp.tile([128, 4], f32)
            nc.sync.dma_start(out=cosk_t[:], in_=cosfk[:])
            nc.sync.dma_start(out=sink_t[:], in_=sinfk[:])
            nc.sync.dma_start(out=cosq_t[:], in_=cosfq[:])
            nc.sync.dma_start(out=sinq_t[:], in_=sinfq[:])
            nc.sync.dma_start(out=perm_t[:], in_=perm_d[:])
            nc.sync.dma_start(out=ident_t[:], in_=ident_d[:])
            nc.sync.dma_start(out=band_t[:], in_=band_d[:])
            nc.sync.dma_start(out=zcor_t[:], in_=zcor_d[:])

            with tc.tile_pool(name="xkvp", bufs=1) as xp:
                xkv_t = xp.tile([128, NT, KBUF], f32r)
                for hi in range(NT):
                    nc.sync.dma_start(
                        out=xkv_t[:, hi, :],
                        in_=xkvT[hi * 128 : (hi + 1) * 128, :],
                    )

                # ---- v projection (all 4 kv heads at once, t-tile major) ----
                with (
                    tc.tile_pool(name="wvp", bufs=1) as wvp,
                    tc.tile_pool(name="vps", bufs=1, space="PSUM") as vps,
                ):
                    wv_t = wvp.tile([128, NT, 512], f32r)
                    for hi in range(NT):
                        nc.sync.dma_start(
                            out=wv_t[:, hi, :],
                            in_=wvT[hi * 128 : (hi + 1) * 128, :],
                        )
                    for ts in range(8):
                        pv = vps.tile([128, 512], f32, tag="pv", bufs=4)
                        for hi in range(NT):
                            nc.tensor.matmul(
                                pv[:],
                                xkv_t[:, hi, ts * 128 : (ts + 1) * 128],
                                wv_t[:, hi, :],
                                start=(hi == 0),
                                stop=(hi == NT - 1),
                            )
                        nc.scalar.copy(out=v_all[:, ts, :], in_=pv[:])

                # ---- per kv-group: k proj+rope, q proj+rope, attention ----
                with (
                    tc.tile_pool(name="gw", bufs=1) as gw,
                    tc.tile_pool(name="gact", bufs=1) as ga,
                    tc.tile_pool(name="gps", bufs=1, space="PSUM") as gp,
                ):
                    for g in range(KV_HEADS):
                        # -- k projection: kT [d=128, t=KBUF] --
                        wk_g = gw.tile([128, NT, 128], f32r, tag="wk", bufs=2)
                        nc.sync.dma_start(
                            out=wk_g[:],
                            in_=wkT.rearrange("(a p) o -> p a o", p=128)[
                                :, :, g * 128 : (g + 1) * 128
                            ],
                        )
                        pk = gp.tile([128, 1024], f32, tag="big", bufs=2)
                        for th in range(2):
                            sl = slice(th * 512, (th + 1) * 512)
                            for hi in range(NT):
                                nc.tensor.matmul(
                                    pk[:, sl],
                                    wk_g[:, hi, :],
                                    xkv_t[:, hi, sl],
                                    start=(hi == 0),
                                    stop=(hi == NT - 1),
                                )
                        kraw = ga.tile([128, 1024], f32r, tag="raw", bufs=2)
                        nc.scalar.copy(out=kraw[:], in_=pk[:])
                        khat = gp.tile([128, 1024], f32, tag="big", bufs=2)
                        for th in range(2):
                            sl = slice(th * 512, (th + 1) * 512)
                            nc.tensor.matmul(
                                khat[:, sl], perm_t[:], kraw[:, sl],
                                start=True, stop=True,
                            )
                        kT_g = ga.tile([128, KBUF], f32r, tag="kT", bufs=1)
                        atmp = ga.tile([128, 1024], f32, tag="tmp", bufs=2)
                        nc.vector.tensor_tensor(
                            out=atmp[:], in0=kraw[:], in1=cosk_t[:], op=mult
                        )
                        nc.vector.tensor_tensor(
                            out=kT_g[:], in0=khat[:], in1=sink_t[:], op=mult
                        )
                        nc.vector.tensor_tensor(
                            out=kT_g[:], in0=kT_g[:], in1=atmp[:], op=add
                        )

                        # -- q projection for heads 4g..4g+3, 2 pairs, hi-outer --
                        pqs = [
                            gp.tile([128, 1024], f32, tag="big", bufs=2,
                                    name=f"pq_{g}_{p}")
                            for p in range(2)
                        ]
                        for hi in range(NT):
                            wq_t = gw.tile(
                                [128, 512], f32r, tag="wq", bufs=4,
                                name=f"wq_{g}_{hi}",
                            )
                            nc.sync.dma_start(
                                out=wq_t[:],
                                in_=wqT[
                                    hi * 128 : (hi + 1) * 128,
                                    g * 512 : (g + 1) * 512,
                                ],
                            )
                            for pair in range(2):
                                for j in range(2):
                                    sl = slice(j * 512, (j + 1) * 512)
                                    nc.tensor.matmul(
                                        pqs[pair][:, sl],
                                        wq_t[:, (2 * pair + j) * 128
                                             : (2 * pair + j + 1) * 128],
                                        xkv_t[:, hi, 512:1024],
                                        start=(hi == 0),
                                        stop=(hi == NT - 1),
                                    )
                        qT_g = ga.tile([128, 2, 1024], f32r, tag="qT", bufs=1)
                        for pair in range(2):
                            qraw = ga.tile([128, 1024], f32r, tag="raw", bufs=2,
                                           name=f"qraw_{g}_{pair}")
                            nc.scalar.copy(out=qraw[:], in_=pqs[pair][:])
                            qhat = gp.tile([128, 1024], f32, tag="big", bufs=2,
                                           name=f"qhat_{g}_{pair}")
                            for j in range(2):
                                sl = slice(j * 512, (j + 1) * 512)
                                nc.tensor.matmul(
                                    qhat[:, sl], perm_t[:], qraw[:, sl],
                                    start=True, stop=True,
                                )
                            btmp = ga.tile([128, 1024], f32, tag="tmp", bufs=2,
                                           name=f"btmp_{g}_{pair}")
                            nc.vector.tensor_tensor(
                                out=btmp[:], in0=qraw[:], in1=cosq_t[:], op=mult
                            )
                            nc.vector.tensor_tensor(
                                out=qT_g[:, pair, :], in0=qhat[:], in1=sinq_t[:],
                                op=mult,
                            )
                            nc.vector.tensor_tensor(
                                out=qT_g[:, pair, :], in0=qT_g[:, pair, :],
                                in1=btmp[:], op=add,
                            )

                        # -- attention for the 4 heads of this group --
                        for hl in range(REPS):
                            qh = 4 * g + hl
                            pair, j = hl // 2, hl % 2
                            for qt in range(4):
                                s_ps = gp.tile(
                                    [128, 640], f32, tag="S", bufs=2,
                                    name=f"s_ps_{qh}_{qt}",
                                )
                                q_sl = qT_g[
                                    :, pair,
                                    j * 512 + qt * 128 : j * 512 + (qt + 1) * 128,
                                ]
                                nc.tensor.matmul(
                                    s_ps[:, 0:512],
                                    q_sl,
                                    kT_g[:, qt * 128 : qt * 128 + 512],
                                    start=True,
                                    stop=True,
                                )
                                nc.tensor.matmul(
                                    s_ps[:, 512:640],
                                    q_sl,
                                    kT_g[:, qt * 128 + 512 : qt * 128 + 640],
                                    start=True,
                                    stop=True,
                                )
                                # band mask on first/last 128-col blocks
                                s_v = s_ps[:].rearrange("p (a b) -> p a b", b=128)
                                nc.vector.tensor_tensor(
                                    out=s_v[:, 0::4, :],
                                    in0=s_v[:, 0::4, :],
                                    in1=band_t[:],
                                    op=add,
                                )
                                p_sb = ga.tile(
                                    [128, 640], f32r, tag="P", bufs=3,
                                    name=f"p_sb_{qh}_{qt}",
                                )
                                z_h = ga.tile([128, 1], f32, tag="z", bufs=3,
                                              name=f"z_{qh}_{qt}")
                                nc.scalar.activation(
                                    out=p_sb[:], in_=s_ps[:], func=Exp,
                                    accum_out=z_h[:],
                                )
                                r_h = ga.tile([128, 1], f32, tag="r", bufs=3,
                                              name=f"r_{qh}_{qt}")
                                nc.vector.tensor_tensor(
                                    out=r_h[:], in0=z_h[:],
                                    in1=zcor_t[:, qt : qt + 1], op=subtract
                                )
                                nc.vector.reciprocal(out=r_h[:], in_=r_h[:])
                                nc.vector.tensor_scalar(
                                    out=p_sb[:], in0=p_sb[:],
                                    scalar1=r_h[:], scalar2=None,
                                    op0=mult,
                                )
                                pt_ps = gp.tile(
                                    [128, 5, 128], f32r, tag="S", bufs=2,
                                    name=f"pt_ps_{qh}_{qt}",
                                )
                                for i in range(5):
                                    nc.tensor.transpose(
                                        pt_ps[:, i, :],
                                        p_sb[:, i * 128 : (i + 1) * 128],
                                        ident_t[:],
                                    )
                                pt_sb = ga.tile(
                                    [128, 5, 128], f32r, tag="PT", bufs=2,
                                    name=f"pt_sb_{qh}_{qt}",
                                )
                                nc.scalar.copy(out=pt_sb[:], in_=pt_ps[:])
                                o_ps = gp.tile(
                                    [128, 128], f32, tag="S", bufs=2,
                                    name=f"o_ps_{qh}_{qt}",
                                )
                                for i in range(5):
                                    nc.tensor.matmul(
                                        o_ps[:],
                                        v_all[:, qt + i, g * 128 : (g + 1) * 128],
                                        pt_sb[:, i, :],
                                        start=(i == 0),
                                        stop=(i == 4),
                                    )
                                nc.scalar.copy(
                                    out=oT_all[:, qh, qt * 128 : (qt + 1) * 128],
                                    in_=o_ps[:],
                                )

            # ---- o_proj: y[t, :] = sum_o oT[o, t] * woT[o, :] ----
            with (
                tc.tile_pool(name="wop", bufs=1) as wop,
                tc.tile_pool(name="yp", bufs=1) as yp,
                tc.tile_pool(name="ops", bufs=1, space="PSUM") as ops,
            ):
                for half in range(2):
                    pys = [
                        ops.tile([128, 1024], f32, tag="py", bufs=4,
                                 name=f"py_{half}_{qt}")
                        for qt in range(4)
                    ]
                    for oh in range(NT):
                        wo_t = wop.tile([128, 1024], f32r, tag="wo", bufs=5,
                                        name=f"wo_{half}_{oh}")
                        nc.sync.dma_start(
                            out=wo_t[:],
                            in_=woT[
                                oh * 128 : (oh + 1) * 128,
                                half * 1024 : (half + 1) * 1024,
                            ],
                        )
                        for qt in range(4):
                            for c in range(2):
                                sl = slice(c * 512, (c + 1) * 512)
                                nc.tensor.matmul(
                                    pys[qt][:, sl],
                                    oT_all[:, oh, qt * 128 : (qt + 1) * 128],
                                    wo_t[:, sl],
                                    start=(oh == 0),
                                    stop=(oh == NT - 1),
                                )
                    for qt in range(4):
                        ysb = yp.tile([128, 1024], f32, tag="y", bufs=4,
                                      name=f"y_{half}_{qt}")
                        nc.scalar.copy(out=ysb[:], in_=pys[qt][:])
                        nc.sync.dma_start(
                            out=y[
                                qt * 128 : (qt + 1) * 128,
                                half * 1024 : (half + 1) * 1024,
                            ],
                            in_=ysb[:],
                        )

    nc.compile()
    return nc


def _host_prep(x, wq, wk, wv, wo):
    x = np.asarray(x, dtype=np.float32)
    B, T, H = x.shape
    assert (B, T, H) == (1, T_FULL, HIDDEN)
    xf = x[0]

    inv_freq = (
        1.0 / (ROPE_BASE ** (np.arange(HALF, dtype=np.float32) / HALF))
    ).astype(np.float32)
    t_all = np.arange(T_FULL, dtype=np.float32)
    freqs = np.outer(t_all, inv_freq).astype(np.float32)  # [T, 64]
    cos_all = np.cos(freqs).astype(np.float32)
    sin_all = np.sin(freqs).astype(np.float32)

    wqT = np.ascontiguousarray(np.asarray(wq, np.float32).T)
    wkT = np.ascontiguousarray(np.asarray(wk, np.float32).T)
    wvT = np.ascontiguousarray(np.asarray(wv, np.float32).T)
    woT = np.ascontiguousarray(np.asarray(wo, np.float32).T)

    perm = np.zeros((128, 128), np.float32)
    for d in range(HALF):
        perm[HALF + d, d] = -1.0  # khat[d] = -k[d+64]
        perm[d, HALF + d] = 1.0  # khat[d+64] = k[d]
    ident = np.eye(128, dtype=np.float32)

    band = np.zeros((128, 2, 128), np.float32)
    r = np.arange(128)[:, None]
    jj = np.arange(128)[None, :]
    band[:, 0, :] = np.where(jj >= r, 0.0, NEG)
    band[:, 1, :] = np.where(jj <= r, 0.0, NEG)

    in_maps = []
    for c in range(NCORES):
        S = c * CHUNK
        lo = S - WINDOW
        xkv = np.zeros((KBUF, HIDDEN), np.float32)
        ck = np.ones((KBUF, HALF), np.float32)  # cos=1 where pos undefined
        sk = np.zeros((KBUF, HALF), np.float32)
        src_lo = max(0, lo)
        off = src_lo - lo
        xkv[off:] = xf[src_lo : S + CHUNK]
        ck[off:] = cos_all[src_lo : S + CHUNK]
        sk[off:] = sin_all[src_lo : S + CHUNK]

        cosfk = np.concatenate([ck.T, ck.T], axis=0)  # [128, KBUF]
        sinfk = np.concatenate([sk.T, sk.T], axis=0)
        cq = (cos_all[S : S + CHUNK] * SCALE).astype(np.float32)
        sq = (sin_all[S : S + CHUNK] * SCALE).astype(np.float32)
        cosfq1 = np.concatenate([cq.T, cq.T], axis=0)  # [128, CHUNK]
        sinfq1 = np.concatenate([sq.T, sq.T], axis=0)
        cosfq = np.concatenate([cosfq1, cosfq1], axis=1)  # [128, 2*CHUNK]
        sinfq = np.concatenate([sinfq1, sinfq1], axis=1)

        zcor = np.zeros((128, 4), np.float32)
        if c == 0:
            i = (np.arange(4) * 128)[None, :] + np.arange(128)[:, None]
            zcor[:] = np.maximum(0, WINDOW - i)

        in_maps.append(
            {
                "xkvT": np.ascontiguousarray(xkv.T),
                "wqT": wqT,
                "wkT": wkT,
                "wvT": wvT,
                "woT": woT,
                "cosfk": np.ascontiguousarray(cosfk),
                "sinfk": np.ascontiguousarray(sinfk),
                "cosfq": np.ascontiguousarray(cosfq),
                "sinfq": np.ascontiguousarray(sinfq),
                "perm": perm,
                "ident": ident,
                "band": band,
                "zcor": zcor,
            }
        )
    return in_maps


def kernel(x, wq, wk, wv, wo):
    global LAST_RESULTS
    from concourse.bass_utils import run_bass_kernel_spmd

    if os.environ.get("BASS_TRACE"):
        _install_profhook()
    if "nc" not in _CACHE:
        _CACHE["nc"] = _build()
    nc = _CACHE["nc"]
    in_maps = _host_prep(x, wq, wk, wv, wo)
    res = run_bass_kernel_spmd(nc, in_maps, core_ids=list(range(NCORES)))
    LAST_RESULTS = res
    out = np.concatenate([res.results[c]["y"] for c in range(NCORES)], axis=0)
    return out[None].astype(np.float32)


# revision 10
# speedup vs baseline: 2.6370x; 1.4231x over previous
"""Local-window GQA attention on 8 trn2 NeuronCores.

Sharding: sequence-parallel. Core c owns queries [c*512, (c+1)*512) and
redundantly computes K/V for its 1024-position key buffer
[c*512-512, c*512+512) from x (halo recompute instead of any collective).
All projections, RoPE, banded-causal softmax and o_proj run on-device in
fp32/fp32r; host only shards/transposes inputs and concatenates outputs.

Device-side layouts are transposed ([feature, time]) so every matmul
contraction lands on the partition axis without on-chip transposes of
activations; only the post-softmax P tiles are transposed (PE transpose).
RoPE rotate-half is a PERM-matrix matmul (DVE cannot cross partitions);
softmax runs unnormalized (scaled scores stay within +-~8 so exp cannot
overflow) with the normalization folded into P, and core 0's halo keys are
handled by zeroed x + a per-core additive Z correction (halo keys give
exp(0)=1, subtracted from Z; their v contribution is 0).
"""

import os

import numpy as np

Q_HEADS, KV_HEADS, HEAD_DIM, WINDOW = 16, 4, 128, 512
HIDDEN = 2048
T_FULL = 4096
NCORES = 8
CHUNK = T_FULL // NCORES  # 512
KBUF = CHUNK + WINDOW  # 1024
REPS = Q_HEADS // KV_HEADS  # 4
HALF = HEAD_DIM // 2  # 64
SCALE = 1.0 / np.sqrt(HEAD_DIM)
NEG = -30000.0
ROPE_BASE = 10000.0

_CACHE = {}
LAST_RESULTS = None


def _install_profhook():
    """Make run_bass_kernel_spmd(trace=True) work under axon (the agent
    image's antenv lacks axon_hooks; register the ctypes NTFF hook)."""
    import sys
    import types

    if "antenv.axon_hooks" in sys.modules:
        return
    m = types.ModuleType("antenv.axon_hooks")
    hook = [None]
    m.set_axon_ntff_profile_hook = lambda h: hook.__setitem__(0, h)
    m.get_axon_ntff_profile_hook = lambda: hook[0]
    sys.modules["antenv.axon_hooks"] = m
    try:
        import antenv

        antenv.axon_hooks = m
        from trn_agent_boot.trn_boot import _ntff_profile_via_ctypes

        m.set_axon_ntff_profile_hook(
            _ntff_profile_via_ctypes("/opt/axon/libaxon_pjrt.so")
        )
    except Exception:
        pass


def _build():
    import concourse.bacc as bacc
    import concourse.mybir as mybir
    from concourse import tile

    f32 = mybir.dt.float32
    f32r = mybir.dt.float32r
    Exp = mybir.ActivationFunctionType.Exp
    mult = mybir.AluOpType.mult
    add = mybir.AluOpType.add
    subtract = mybir.AluOpType.subtract

    nc = bacc.Bacc(None, target_bir_lowering=False)

    # ---- DRAM I/O (per core) ----
    xkvT = nc.dram_tensor("xkvT", [HIDDEN, KBUF], f32r, kind="ExternalInput")
    wqT = nc.dram_tensor("wqT", [HIDDEN, 2048], f32r, kind="ExternalInput")
    wkT = nc.dram_tensor("wkT", [HIDDEN, 512], f32r, kind="ExternalInput")
    wvT = nc.dram_tensor("wvT", [HIDDEN, 512], f32r, kind="ExternalInput")
    woT = nc.dram_tensor("woT", [2048, 2048], f32r, kind="ExternalInput")
    cosfk = nc.dram_tensor("cosfk", [128, KBUF], f32, kind="ExternalInput")
    sinfk = nc.dram_tensor("sinfk", [128, KBUF], f32, kind="ExternalInput")
    cosfq = nc.dram_tensor("cosfq", [128, 2 * CHUNK], f32, kind="ExternalInput")
    sinfq = nc.dram_tensor("sinfq", [128, 2 * CHUNK], f32, kind="ExternalInput")
    perm_d = nc.dram_tensor("perm", [128, 128], f32r, kind="ExternalInput")
    ident_d = nc.dram_tensor("ident", [128, 128], f32r, kind="ExternalInput")
    band_d = nc.dram_tensor("band", [128, 2, 128], f32, kind="ExternalInput")
    zcor_d = nc.dram_tensor("zcor", [128, 4], f32, kind="ExternalInput")
    y = nc.dram_tensor("y", [CHUNK, HIDDEN], f32, kind="ExternalOutput")

    NT = HIDDEN // 128  # 16 h-tiles

    with tile.TileContext(nc) as tc:
        with (
            tc.tile_pool(name="persist", bufs=1) as pp,
            tc.tile_pool(name="consts", bufs=1) as cp,
        ):
            # persistent activations
            v_all = pp.tile([128, 8, 512], f32r)  # [s, s-tile, d(4 heads)]
            oT_all = pp.tile([128, 16, 512], f32r)  # [d, head, t]
            # constants
            cosk_t = cp.tile([128, KBUF], f32)
            sink_t = cp.tile([128, KBUF], f32)
            cosq_t = cp.tile([128, 2 * CHUNK], f32)
            sinq_t = cp.tile([128, 2 * CHUNK], f32)
            perm_t = cp.tile([128, 128], f32r)
            ident_t = cp.tile([128, 128], f32r)
            band_t = cp.tile([128, 2, 128], f32)
            zcor_t = cp.tile([128, 4], f32)
            nc.sync.dma_start(out=cosk_t[:], in_=cosfk[:])
            nc.sync.dma_start(out=sink_t[:], in_=sinfk[:])
            nc.sync.dma_start(out=cosq_t[:], in_=cosfq[:])
            nc.sync.dma_start(out=sinq_t[:], in_=sinfq[:])
            nc.sync.dma_start(out=perm_t[:], in_=perm_d[:])
            nc.sync.dma_start(out=ident_t[:], in_=ident_d[:])
            nc.sync.dma_start(out=band_t[:], in_=band_d[:])
            nc.sync.dma_start(out=zcor_t[:], in_=zcor_d[:])

            with tc.tile_pool(name="xkvp", bufs=1) as xp:
                xkv_t = xp.tile([128, NT, KBUF], f32r)
                for hi in range(NT):
                    nc.sync.dma_start(
                        out=xkv_t[:, hi, :],
                        in_=xkvT[hi * 128 : (hi + 1) * 128, :],
                    )

                # ---- v projection (all 4 kv heads at once, t-tile major) ----
                with (
                    tc.tile_pool(name="wvp", bufs=1) as wvp,
                    tc.tile_pool(name="vps", bufs=1, space="PSUM") as vps,
                ):
                    wv_t = wvp.tile([128, NT, 512], f32r)
                    for hi in range(NT):
                        nc.sync.dma_start(
                            out=wv_t[:, hi, :],
                            in_=wvT[hi * 128 : (hi + 1) * 128, :],
                        )
                    for ts in range(8):
                        pv = vps.tile([128, 512], f32, tag="pv", bufs=4)
                        for hi in range(NT):
                            nc.tensor.matmul(
                                pv[:],
                                xkv_t[:, hi, ts * 128 : (ts + 1) * 128],
                                wv_t[:, hi, :],
                                start=(hi == 0),
                                stop=(hi == NT - 1),
                            )
                        nc.scalar.copy(out=v_all[:, ts, :], in_=pv[:])

                # ---- per kv-group: k proj+rope, q proj+rope, attention ----
                with (
                    tc.tile_pool(name="gw", bufs=1) as gw,
                    tc.tile_pool(name="gact", bufs=1) as ga,
                    tc.tile_pool(name="gps", bufs=1, space="PSUM") as gp,
                ):
                    for g in range(KV_HEADS):
                        # -- k projection: kT [d=128, t=KBUF] --
                        wk_g = gw.tile([128, NT, 128], f32r, tag="wk", bufs=1)
                        nc.sync.dma_start(
                            out=wk_g[:],
                            in_=wkT.rearrange("(a p) o -> p a o", p=128)[
                                :, :, g * 128 : (g + 1) * 128
                            ],
                        )
                        pk = gp.tile([128, 1024], f32, tag="W", bufs=4)
                        for th in range(2):
                            sl = slice(th * 512, (th + 1) * 512)
                            for hi in range(NT):
                                nc.tensor.matmul(
                                    pk[:, sl],
                                    wk_g[:, hi, :],
                                    xkv_t[:, hi, sl],
                                    start=(hi == 0),
                                    stop=(hi == NT - 1),
                                )
                        kraw = ga.tile([128, 1024], f32r, tag="raw", bufs=2)
                        nc.scalar.copy(out=kraw[:], in_=pk[:])
                        khat = gp.tile([128, 1024], f32, tag="W", bufs=4)
                        for th in range(2):
                            sl = slice(th * 512, (th + 1) * 512)
                            nc.tensor.matmul(
                                khat[:, sl], perm_t[:], kraw[:, sl],
                                start=True, stop=True,
                            )
                        kT_g = ga.tile([128, KBUF], f32r, tag="kT", bufs=1)
                        atmp = ga.tile([128, 1024], f32, tag="tmp", bufs=2)
                        nc.vector.tensor_tensor(
                            out=atmp[:], in0=kraw[:], in1=cosk_t[:], op=mult
                        )
                        nc.vector.tensor_tensor(
                            out=kT_g[:], in0=khat[:], in1=sink_t[:], op=mult
                        )
                        nc.vector.tensor_tensor(
                            out=kT_g[:], in0=kT_g[:], in1=atmp[:], op=add
                        )

                        # -- q projection for heads 4g..4g+3, 2 pairs, hi-outer --
                        pqs = [
                            gp.tile([128, 1024], f32, tag="W", bufs=4,
                                    name=f"pq_{g}_{p}")
                            for p in range(2)
                        ]
                        for hi in range(NT):
                            wq_t = gw.tile(
                                [128, 512], f32r, tag="wq", bufs=4,
                                name=f"wq_{g}_{hi}",
                            )
                            nc.sync.dma_start(
                                out=wq_t[:],
                                in_=wqT[
                                    hi * 128 : (hi + 1) * 128,
                                    g * 512 : (g + 1) * 512,
                                ],
                            )
                            for pair in range(2):
                                for j in range(2):
                                    sl = slice(j * 512, (j + 1) * 512)
                                    nc.tensor.matmul(
                                        pqs[pair][:, sl],
                                        wq_t[:, (2 * pair + j) * 128
                                             : (2 * pair + j + 1) * 128],
                                        xkv_t[:, hi, 512:1024],
                                        start=(hi == 0),
                                        stop=(hi == NT - 1),
                                    )
                        qT_g = ga.tile([128, 2, 1024], f32r, tag="qT", bufs=1)
                        for pair in range(2):
                            qraw = ga.tile([128, 1024], f32r, tag="raw", bufs=2,
                                           name=f"qraw_{g}_{pair}")
                            nc.scalar.copy(out=qraw[:], in_=pqs[pair][:])
                            qhat = gp.tile([128, 1024], f32, tag="W", bufs=4,
                                           name=f"qhat_{g}_{pair}")
                            for j in range(2):
                                sl = slice(j * 512, (j + 1) * 512)
                                nc.tensor.matmul(
                                    qhat[:, sl], perm_t[:], qraw[:, sl],
                                    start=True, stop=True,
                                )
                            btmp = ga.tile([128, 1024], f32, tag="tmp", bufs=2,
                                           name=f"btmp_{g}_{pair}")
                            nc.vector.tensor_tensor(
                                out=btmp[:], in0=qraw[:], in1=cosq_t[:], op=mult
                            )
                            nc.vector.tensor_tensor(
                                out=qT_g[:, pair, :], in0=qhat[:], in1=sinq_t[:],
                                op=mult,
                            )
                            nc.vector.tensor_tensor(
                                out=qT_g[:, pair, :], in0=qT_g[:, pair, :],
                                in1=btmp[:], op=add,
                            )

                        # -- attention for the 4 heads of this group --
                        for hl in range(REPS):
                            qh = 4 * g + hl
                            pair, j = hl // 2, hl % 2
                            z_h = ga.tile([128, 4], f32, tag="z", bufs=2,
                                          name=f"z_{qh}")
                            p_sbs = []
                            for qt in range(4):
                                s_ps = gp.tile(
                                    [128, 640], f32, tag="W", bufs=4,
                                    name=f"s_ps_{qh}_{qt}",
                                )
                                q_sl = qT_g[
                                    :, pair,
                                    j * 512 + qt * 128 : j * 512 + (qt + 1) * 128,
                                ]
                                nc.tensor.matmul(
                                    s_ps[:, 0:512],
                                    q_sl,
                                    kT_g[:, qt * 128 : qt * 128 + 512],
                                    start=True,
                                    stop=True,
                                )
                                nc.tensor.matmul(
                                    s_ps[:, 512:640],
                                    q_sl,
                                    kT_g[:, qt * 128 + 512 : qt * 128 + 640],
                                    start=True,
                                    stop=True,
                                )
                                # band mask on first/last 128-col blocks
                                s_v = s_ps[:].rearrange("p (a b) -> p a b", b=128)
                                nc.vector.tensor_tensor(
                                    out=s_v[:, 0::4, :],
                                    in0=s_v[:, 0::4, :],
                                    in1=band_t[:],
                                    op=add,
                                )
                                p_sb = ga.tile(
                                    [128, 640], f32r, tag="P", bufs=4,
                                    name=f"p_sb_{qh}_{qt}",
                                )
                                nc.scalar.activation(
                                    out=p_sb[:], in_=s_ps[:], func=Exp,
                                    accum_out=z_h[:, qt : qt + 1],
                                )
                                p_sbs.append(p_sb)
                            r_h = ga.tile([128, 4], f32, tag="r", bufs=2,
                                          name=f"r_{qh}")
                            nc.vector.tensor_tensor(
                                out=r_h[:], in0=z_h[:], in1=zcor_t[:], op=subtract
                            )
                            nc.vector.reciprocal(out=r_h[:], in_=r_h[:])
                            # PT_head[:, b, c*128:...] holds PT columns for
                            # query-tile qt = a_b + c of key-block b
                            pt_head = ga.tile([128, 8, 512], f32r, tag="PTH",
                                              bufs=1, name=f"pt_head_{qh}")
                            for qt in range(4):
                                p_sb = p_sbs[qt]
                                nc.vector.tensor_scalar(
                                    out=p_sb[:], in0=p_sb[:],
                                    scalar1=r_h[:, qt : qt + 1], scalar2=None,
                                    op0=mult,
                                )
                                pt_ps = gp.tile(
                                    [128, 5, 128], f32r, tag="W", bufs=4,
                                    name=f"pt_ps_{qh}_{qt}",
                                )
                                for i in range(5):
                                    nc.tensor.transpose(
                                        pt_ps[:, i, :],
                                        p_sb[:, i * 128 : (i + 1) * 128],
                                        ident_t[:],
                                    )
                                # scatter the 5 blocks into pt_head:
                                # block b = qt+i; dest col = qt*128 (b<=3)
                                #                         or (4-i)*128 (b>=4)
                                n1 = min(5, 4 - qt)  # i range with b <= 3
                                ph_flat = pt_head[:].rearrange("p a b -> p (a b)")
                                if n1 > 0:
                                    src_ap = pt_ps[:, 0:n1, :]
                                    # dest offsets: (qt+i)*512 + qt*128, i=0..n1-1
                                    d0 = qt * 512 + qt * 128
                                    dst_ap = ph_flat[
                                        :, d0 : d0 + (n1 - 1) * 512 + 128
                                    ].rearrange(
                                        "p (a b) -> p a b", b=128
                                    )[:, 0::4, :] if n1 > 1 else ph_flat[
                                        :, d0 : d0 + 128
                                    ]
                                    nc.scalar.copy(out=dst_ap, in_=src_ap)
                                if n1 < 5:
                                    # part2: i = n1..4, dest = qt*512+512+384*i
                                    i0 = n1
                                    cnt = 5 - n1
                                    d0 = qt * 512 + 512 + 384 * i0
                                    src_ap = pt_ps[:, i0:5, :]
                                    dst_ap = ph_flat[
                                        :, d0 : d0 + (cnt - 1) * 384 + 128
                                    ].rearrange(
                                        "p (a b) -> p a b", b=128
                                    )[:, 0::3, :] if cnt > 1 else ph_flat[
                                        :, d0 : d0 + 128
                                    ]
                                    nc.scalar.copy(out=dst_ap, in_=src_ap)
                            # PV: one (or two) wide matmuls per key block
                            o_ps = gp.tile([128, 512], f32, tag="W", bufs=4,
                                           name=f"o_ps_{qh}")
                            for b in range(8):
                                a_b = max(0, b - 4)
                                v_sl = v_all[:, b, g * 128 : (g + 1) * 128]
                                if b <= 3:
                                    # start=True only on the bank's first MM:
                                    # the has_written clear is bank-granular
                                    nc.tensor.matmul(
                                        o_ps[:, b * 128 : (b + 1) * 128],
                                        v_sl,
                                        pt_head[:, b,
                                                (b - a_b) * 128
                                                : (b - a_b + 1) * 128],
                                        start=(b == 0),
                                        stop=False,
                                        skip_group_check=True,
                                    )
                                hi_q = min(b, 4)
                                if hi_q > a_b:
                                    nc.tensor.matmul(
                                        o_ps[:, a_b * 128 : hi_q * 128],
                                        v_sl,
                                        pt_head[:, b, 0 : (hi_q - a_b) * 128],
                                        start=False,
                                        stop=(b == 7),
                                        skip_group_check=True,
                                    )
                            nc.scalar.copy(out=oT_all[:, qh, :], in_=o_ps[:])

            # ---- o_proj: y[t, :] = sum_o oT[o, t] * woT[o, :] ----
            with (
                tc.tile_pool(name="wop", bufs=1) as wop,
                tc.tile_pool(name="yp", bufs=1) as yp,
                tc.tile_pool(name="ops", bufs=1, space="PSUM") as ops,
            ):
                for half in range(2):
                    pys = [
                        ops.tile([128, 1024], f32, tag="py", bufs=4,
                                 name=f"py_{half}_{qt}")
                        for qt in range(4)
                    ]
                    for oh in range(NT):
                        wo_t = wop.tile([128, 1024], f32r, tag="wo", bufs=5,
                                        name=f"wo_{half}_{oh}")
                        nc.sync.dma_start(
                            out=wo_t[:],
                            in_=woT[
                                oh * 128 : (oh + 1) * 128,
                                half * 1024 : (half + 1) * 1024,
                            ],
                        )
                        for qt in range(4):
                            for c in range(2):
                                sl = slice(c * 512, (c + 1) * 512)
                                nc.tensor.matmul(
                                    pys[qt][:, sl],
                                    oT_all[:, oh, qt * 128 : (qt + 1) * 128],
                                    wo_t[:, sl],
                                    start=(oh == 0),
                                    stop=(oh == NT - 1),
                                )
                    for qt in range(4):
                        ysb = yp.tile([128, 1024], f32, tag="y", bufs=4,
                                      name=f"y_{half}_{qt}")
                        nc.scalar.copy(out=ysb[:], in_=pys[qt][:])
                        nc.sync.dma_start(
                            out=y[
                                qt * 128 : (qt + 1) * 128,
                                half * 1024 : (half + 1) * 1024,
                            ],
                            in_=ysb[:],
                        )

    nc.compile()
    return nc


def _host_prep(x, wq, wk, wv, wo):
    x = np.asarray(x, dtype=np.float32)
    B, T, H = x.shape
    assert (B, T, H) == (1, T_FULL, HIDDEN)
    xf = x[0]

    inv_freq = (
        1.0 / (ROPE_BASE ** (np.arange(HALF, dtype=np.float32) / HALF))
    ).astype(np.float32)
    t_all = np.arange(T_FULL, dtype=np.float32)
    freqs = np.outer(t_all, inv_freq).astype(np.float32)  # [T, 64]
    cos_all = np.cos(freqs).astype(np.float32)
    sin_all = np.sin(freqs).astype(np.float32)

    wqT = np.ascontiguousarray(np.asarray(wq, np.float32).T)
    wkT = np.ascontiguousarray(np.asarray(wk, np.float32).T)
    wvT = np.ascontiguousarray(np.asarray(wv, np.float32).T)
    woT = np.ascontiguousarray(np.asarray(wo, np.float32).T)

    perm = np.zeros((128, 128), np.float32)
    for d in range(HALF):
        perm[HALF + d, d] = -1.0  # khat[d] = -k[d+64]
        perm[d, HALF + d] = 1.0  # khat[d+64] = k[d]
    ident = np.eye(128, dtype=np.float32)

    band = np.zeros((128, 2, 128), np.float32)
    r = np.arange(128)[:, None]
    jj = np.arange(128)[None, :]
    band[:, 0, :] = np.where(jj >= r, 0.0, NEG)
    band[:, 1, :] = np.where(jj <= r, 0.0, NEG)

    in_maps = []
    for c in range(NCORES):
        S = c * CHUNK
        lo = S - WINDOW
        xkv = np.zeros((KBUF, HIDDEN), np.float32)
        ck = np.ones((KBUF, HALF), np.float32)  # cos=1 where pos undefined
        sk = np.zeros((KBUF, HALF), np.float32)
        src_lo = max(0, lo)
        off = src_lo - lo
        xkv[off:] = xf[src_lo : S + CHUNK]
        ck[off:] = cos_all[src_lo : S + CHUNK]
        sk[off:] = sin_all[src_lo : S + CHUNK]

        cosfk = np.concatenate([ck.T, ck.T], axis=0)  # [128, KBUF]
        sinfk = np.concatenate([sk.T, sk.T], axis=0)
        cq = (cos_all[S : S + CHUNK] * SCALE).astype(np.float32)
        sq = (sin_all[S : S + CHUNK] * SCALE).astype(np.float32)
        cosfq1 = np.concatenate([cq.T, cq.T], axis=0)  # [128, CHUNK]
        sinfq1 = np.concatenate([sq.T, sq.T], axis=0)
        cosfq = np.concatenate([cosfq1, cosfq1], axis=1)  # [128, 2*CHUNK]
        sinfq = np.concatenate([sinfq1, sinfq1], axis=1)

        zcor = np.zeros((128, 4), np.float32)
        if c == 0:
            i = (np.arange(4) * 128)[None, :] + np.arange(128)[:, None]
            zcor[:] = np.maximum(0, WINDOW - i)

        in_maps.append(
            {
                "xkvT": np.ascontiguousarray(xkv.T),
                "wqT": wqT,
                "wkT": wkT,
                "wvT": wvT,
                "woT": woT,
                "cosfk": np.ascontiguousarray(cosfk),
                "sinfk": np.ascontiguousarray(sinfk),
                "cosfq": np.ascontiguousarray(cosfq),
                "sinfq": np.ascontiguousarray(sinfq),
                "perm": perm,
                "ident": ident,
                "band": band,
                "zcor": zcor,
            }
        )
    return in_maps


def kernel(x, wq, wk, wv, wo):
    global LAST_RESULTS
    from concourse.bass_utils import run_bass_kernel_spmd

    if os.environ.get("BASS_TRACE"):
        _install_profhook()
    if "nc" not in _CACHE:
        _CACHE["nc"] = _build()
    nc = _CACHE["nc"]
    in_maps = _host_prep(x, wq, wk, wv, wo)
    res = run_bass_kernel_spmd(nc, in_maps, core_ids=list(range(NCORES)))
    LAST_RESULTS = res
    out = np.concatenate([res.results[c]["y"] for c in range(NCORES)], axis=0)
    return out[None].astype(np.float32)
